# revision 1
# baseline (speedup 1.0000x reference)
"""Trainium2 Bass kernel for the modified-MDPN dendrite model.

Math per output element (b, i, j, m):
    acc = sum_r log(prod_c u)   with u = atan(10*(x*w - q))/pi + 1.1
        = log(prod_{r,c} u)                     (u > 0 always)
Then 4x4 spatial maxpool, flatten (i_o, j_o, m), fc1(7744->128)+relu,
fc2(128->10).

Device strategy (8 NeuronCores, data parallel over batch, 2 images/core):
  - partitions p = m*8 + cp (m: 16 filters, cp: 8 chunks of 12 output rows;
    12 = 3 pool groups so the 4x4 maxpool never crosses partitions; output
    rows 88..95 are garbage lanes masked later by zero fc1 weights; x is
    zero-padded to 104 rows on host for the halo).
  - per window tap (r, c) of the 81: one ACT Arctan instruction over
    [128, 2*12*88] with per-partition scale=10w, bias=-10q folded into the
    activation's pre-affine (HW arctan is accurate far beyond +-pi/2 -- the
    CoreSim range assert is conservative); one DVE tensor_scalar
    (u = t/pi + 1.1, bf16, 4x mode); one DVE multiply into a running
    product (bf16, 2x mode, ping-pong buffers).
  - ln is monotonic, so the 4x4 maxpool runs on the bf16 *products* (two
    free-dim max-reduces, entirely within partitions thanks to the 12-row
    chunking) and Ln runs on the 16x smaller pooled map. ln of the 81-tap
    product equals the reference's sum-over-rows-of-log-of-column-products
    (u > 0; product stays within e^-42..e^+39, inside bf16/f32 range).
  - the last tap runs per-image so image 0's pool/ln chain overlaps
    image 1's final atan on ACT; tap 0 is split by rows so the first atan
    only waits for the first ~0.25MB of the input DMA.
  - fc1: the pooled map y2 is already [(m,cp) partitions, (b,f2)], so the
    7744-long contraction runs directly as 66 accumulating K=128 matmuls
    (rhs strided over b) against host-permuted, zero-padded bf16 fc1
    weights -- no transpose needed; relu+bias fused on ACT; fc2 is one
    matmul.

Engine balance per core (cost model, validated against HW wall-clock
slope with an on-device repeat loop: ~160us measured per 81-tap body vs
159.7us modeled): ACT ~157us stream (critical path), DVE ~146us,
PE ~7us; total ~176us = startup/exit-barrier + ACT stream + ~3us tail.
Pure atan streaming floor is 142.6us; the gap is the per-instruction
SBUF turnaround (~15us, silicon errata) and the 12-row padding (~12us)
that buys the in-partition pool. Measured dead ends: GPSIMD z-precompute
to batch taps per ACT instruction (solo rate 1.09 cyc/elem but ~12x
degradation under concurrent DVE load from SBUF-port contention),
tap-parity instruction packing (cross-partition product combine costs
more than the amortized bubbles), PSUM activation outputs (drop DVE
perf modes).
"""

import sys

sys.path.insert(0, "/opt/trn_rl_repo")

import ml_dtypes
import numpy as np

import concourse.bacc as bacc
import concourse.mybir as mybir
from concourse import tile
from concourse.bass_utils import run_bass_kernel_spmd

AFT = mybir.ActivationFunctionType
ALU = mybir.AluOpType
F32 = mybir.dt.float32
BF16 = mybir.dt.bfloat16

M = 16          # filters
N = 9           # window side
IMG = 96
S = 88          # sliding-window output side
SP = 22         # pooled side
B = 16          # global batch
NCORES = 8
BL = B // NCORES          # images per core (2)
CP = 8                    # row chunks per image
RP = 12                   # output rows per chunk (12*8 = 96 >= 88)
GI = RP // 4              # pooled row-groups per chunk (3)
HALO = RP + N - 1         # input rows per chunk (20)
ROWS_PAD = RP * (CP - 1) + HALO   # padded input rows (104)
FD = BL * RP * S          # free elems per tap instruction (2112)
F2 = GI * SP              # pooled positions per (partition, image) (66)
PI = float(np.pi)

_CACHE = {}


def _build_nc():
    nc = bacc.Bacc("TRN2", target_bir_lowering=False, debug=False)

    xp = nc.declare_dram_parameter("xp", [128, BL * HALO * IMG], F32, isOutput=False)
    ws = nc.declare_dram_parameter("ws", [128, 81], F32, isOutput=False)
    qs = nc.declare_dram_parameter("qs", [128, 81], F32, isOutput=False)
    w1 = nc.declare_dram_parameter("w1", [128, F2 * 128], BF16, isOutput=False)
    w2 = nc.declare_dram_parameter("w2", [128, 10], F32, isOutput=False)
    b1 = nc.declare_dram_parameter("b1", [128, 1], F32, isOutput=False)
    b2 = nc.declare_dram_parameter("b2", [10, 1], F32, isOutput=False)
    out = nc.declare_dram_parameter("out", [10, BL], F32, isOutput=True)

    with tile.TileContext(nc) as tc:
        with (
            tc.tile_pool(name="consts", bufs=1) as cpool,
            tc.tile_pool(name="work", bufs=3) as wpool,
            tc.tile_pool(name="state", bufs=1) as spool,
            tc.tile_pool(name="psum", bufs=1, space="PSUM") as ppool,
        ):
            xs = cpool.tile([128, BL * HALO * IMG], F32, tag="xs")
            wst = cpool.tile([128, 81], F32, tag="wst")
            qst = cpool.tile([128, 81], F32, tag="qst")
            w1t = cpool.tile([128, F2 * 128], BF16, tag="w1t")
            w2t = cpool.tile([128, 10], F32, tag="w2t")
            b1t = cpool.tile([128, 1], F32, tag="b1t")
            b2t = cpool.tile([10, 1], F32, tag="b2t")

            # DMA order matters: the first atan waits on wst/qst and the
            # r=0 halo rows, so issue the small tensors first, then the
            # first 12 halo rows, then the rest; the big fc1 weights last.
            nc.sync.dma_start(wst[:], ws[:])
            nc.sync.dma_start(qst[:], qs[:])
            xsr = xs[:].rearrange("p (b il j) -> p b il j", b=BL, il=HALO, j=IMG)
            xpr = xp.rearrange("p (b il j) -> p b il j", b=BL, il=HALO, j=IMG)
            for p0 in range(0, 128, 32):
                nc.sync.dma_start(
                    xsr[p0 : p0 + 32, :, 0:6], xpr[p0 : p0 + 32, :, 0:6]
                )
            for p0 in range(0, 128, 32):
                nc.sync.dma_start(
                    xsr[p0 : p0 + 32, :, 6:RP], xpr[p0 : p0 + 32, :, 6:RP]
                )
            for p0 in range(0, 128, 64):
                nc.sync.dma_start(
                    xsr[p0 : p0 + 64, :, RP:HALO], xpr[p0 : p0 + 64, :, RP:HALO]
                )
            nc.sync.dma_start(b1t[:], b1[:])
            nc.sync.dma_start(b2t[:], b2[:])
            nc.sync.dma_start(w2t[:], w2[:])
            nc.sync.dma_start(w1t[:], w1[:])

            xr = xs[:].rearrange("p (b il j) -> p b il j", b=BL, il=HALO, j=IMG)

            rp_tiles = [
                spool.tile([128, FD], BF16, tag="rp0", name="rp0"),
                spool.tile([128, FD], BF16, tag="rp1", name="rp1"),
            ]
            cur = 0
            NSPLIT = 80      # taps [NSPLIT, 81) run per-image for tail overlap
            for t in range(NSPLIT):
                r, c = divmod(t, N)
                xv = xr[:, :, r : r + RP, c : c + S]
                ut = wpool.tile([128, BL, RP, S], BF16, tag="atan")
                if t == 0:
                    # split tap 0 by rows so the first atan only waits for
                    # the first 6 halo rows of the input DMA
                    nc.scalar.activation(
                        ut[:, :, 0:6], xv[:, :, 0:6], AFT.Arctan,
                        bias=qst[:, t : t + 1], scale=wst[:, t : t + 1],
                    )
                    nc.scalar.activation(
                        ut[:, :, 6:RP], xv[:, :, 6:RP], AFT.Arctan,
                        bias=qst[:, t : t + 1], scale=wst[:, t : t + 1],
                    )
                else:
                    nc.scalar.activation(
                        ut[:], xv, AFT.Arctan,
                        bias=qst[:, t : t + 1], scale=wst[:, t : t + 1],
                    )
                uf = ut[:].rearrange("p b il j -> p (b il j)")
                if t == 0:
                    nc.vector.tensor_scalar(
                        rp_tiles[0][:], uf, 1.0 / PI, 1.1, ALU.mult, ALU.add
                    )
                else:
                    un = wpool.tile([128, FD], BF16, tag="un")
                    nc.vector.tensor_scalar(
                        un[:], uf, 1.0 / PI, 1.1, ALU.mult, ALU.add
                    )
                    nxt = 1 - cur
                    nc.vector.tensor_tensor(
                        rp_tiles[nxt][:], rp_tiles[cur][:], un[:], ALU.mult
                    )
                    cur = nxt

            # Last taps run per-image so image b's pool/ln/transpose chain
            # overlaps the other image's remaining atans on ACT.
            # ln is monotonic, so maxpool the bf16 products first and take
            # Ln on the 16x smaller pooled map (saves ACT time and takes
            # Ln off the serial tail).
            FD1 = RP * S
            shared = rp_tiles[cur][:].rearrange("p (b f) -> p b f", b=BL, f=FD1)
            y2u = spool.tile([128, BL * F2], BF16, tag="y2u")
            y2uv = y2u[:].rearrange("p (b f2) -> p b f2", b=BL, f2=F2)
            y2 = spool.tile([128, BL * F2], BF16, tag="y2")
            y2v = y2[:].rearrange("p (b f2) -> p b f2", b=BL, f2=F2)
            p1 = spool.tile([128, BL * RP * SP], BF16, tag="p1")
            p1bv = p1[:].rearrange("p (b f) -> p b f", b=BL, f=RP * SP)
            for b in range(BL):
                rpb = [
                    spool.tile([128, FD1], BF16, tag=f"rpb{b}{i}", name=f"rpb{b}{i}")
                    for i in range(2)
                ]
                bcur = -1          # -1 means "shared tile half"
                for t in range(NSPLIT, 81):
                    r, c = divmod(t, N)
                    xvb = xr[:, b : b + 1, r : r + RP, c : c + S]
                    utb = wpool.tile([128, 1, RP, S], BF16, tag="atanb")
                    nc.scalar.activation(
                        utb[:], xvb, AFT.Arctan,
                        bias=qst[:, t : t + 1], scale=wst[:, t : t + 1],
                    )
                    ufb = utb[:].rearrange("p b il j -> p (b il j)")
                    unb = wpool.tile([128, FD1], BF16, tag="unb")
                    nc.vector.tensor_scalar(
                        unb[:], ufb, 1.0 / PI, 1.1, ALU.mult, ALU.add
                    )
                    src = shared[:, b] if bcur < 0 else rpb[bcur][:]
                    bnxt = (bcur + 1) % 2
                    nc.vector.tensor_tensor(rpb[bnxt][:], src, unb[:], ALU.mult)
                    bcur = bnxt
                final_b = shared[:, b] if bcur < 0 else rpb[bcur][:]

                # maxpool over j (groups of 4), output laid out (f2, ii)
                # with f2 = ig*22 + jo, il = 4*ig + ii
                accv = final_b.rearrange(
                    "p (il jo jj) -> p il jo jj", il=RP, jo=SP, jj=4
                )
                p1w = p1bv[:, b].rearrange(
                    "p (ig jo ii) -> p ig ii jo", ig=GI, jo=SP, ii=4
                )
                nc.vector.tensor_reduce(p1w, accv, mybir.AxisListType.X, ALU.max)

                # maxpool over i (= groups of 4 rows: ii axis, innermost)
                p1i = p1bv[:, b].rearrange("p (f2 ii) -> p f2 ii", f2=F2, ii=4)
                nc.vector.tensor_reduce(
                    y2uv[:, b], p1i, mybir.AxisListType.X, ALU.max
                )

                # dendrite output: ln of the pooled 81-tap product
                nc.scalar.activation(y2v[:, b], y2uv[:, b], AFT.Ln)

            # fc1: y2 is already [(m, cp) partitions, (b, f2)] -- contract
            # the partition dim directly: 66 accumulating matmuls of K=128,
            # rhs strided over b. No transpose needed.
            ph = ppool.tile([128, BL], F32, tag="ph")
            y2f = y2[:].rearrange("p (b f2) -> p f2 b", b=BL, f2=F2)
            for g in range(F2):
                nc.tensor.matmul(
                    ph[:],
                    w1t[:, g * 128 : (g + 1) * 128],
                    y2f[:, g],
                    start=(g == 0),
                    stop=(g == F2 - 1),
                )
            h = spool.tile([128, BL], F32, tag="h")
            nc.scalar.activation(h[:], ph[:], AFT.Relu, bias=b1t[:, 0:1])

            # fc2
            po = ppool.tile([10, BL], F32, tag="po")
            nc.tensor.matmul(po[:], w2t[:, 0:10], h[:], start=True, stop=True)
            osb = spool.tile([10, BL], F32, tag="osb")
            nc.scalar.activation(osb[:], po[:], AFT.Identity, bias=b2t[:, 0:1])
            nc.sync.dma_start(out[:], osb[:])

    nc.compile()
    return nc


def _prep_inputs(x, w, q, fc1_w, fc1_b, fc2_w, fc2_b):
    x = np.asarray(x, np.float32)
    w = np.asarray(w, np.float32)
    q = np.asarray(q, np.float32)
    fc1_w = np.asarray(fc1_w, np.float32)
    fc1_b = np.asarray(fc1_b, np.float32)
    fc2_w = np.asarray(fc2_w, np.float32)
    fc2_b = np.asarray(fc2_b, np.float32)

    xpad = np.zeros((B, ROWS_PAD, IMG), np.float32)
    xpad[:, :IMG, :] = x
    # halo chunks: [B, CP, HALO, IMG]
    xh = np.stack(
        [xpad[:, RP * cp : RP * cp + HALO, :] for cp in range(CP)], axis=1
    )

    ws = np.repeat(10.0 * w.reshape(M, 81), CP, axis=0)          # [128, 81]
    qs = np.repeat(-10.0 * q.reshape(M, 81), CP, axis=0)

    # fc1 weights: w1[f2, (m*8+cp)*128 + n] = fc1_w[n, io*352 + jo*16 + m]
    # with io = 3*cp + ig, f2 = ig*22 + jo; zero where io >= 22 (the
    # garbage pool lanes from the 12-row chunking).
    fw = fc1_w.reshape(128, SP, SP, M)            # [n, io, jo, m]
    a = fw.transpose(1, 2, 3, 0)                  # [io, jo, m, n]
    io_idx = 3 * np.arange(CP)[:, None] + np.arange(GI)[None, :]   # [cp, ig]
    valid = (io_idx < SP).astype(np.float32)
    b6 = a[np.clip(io_idx, 0, SP - 1)]            # [cp, ig, jo, m, n]
    b6 = b6 * valid[:, :, None, None, None]
    # [cp, ig, jo, m, n] -> [(m, cp), (ig, jo), n]
    w1 = np.ascontiguousarray(
        b6.transpose(3, 0, 1, 2, 4).reshape(128, F2 * 128)
    ).astype(ml_dtypes.bfloat16)

    w2 = np.ascontiguousarray(fc2_w.T)            # [128, 10]
    b1 = fc1_b.reshape(128, 1).astype(np.float32)
    b2 = fc2_b.reshape(10, 1).astype(np.float32)

    in_maps = []
    for k in range(NCORES):
        arr = xh[BL * k : BL * k + BL]            # [BL, CP, HALO, IMG]
        xpk = np.broadcast_to(arr[None], (M, BL, CP, HALO, IMG))
        xpk = np.ascontiguousarray(
            xpk.transpose(0, 2, 1, 3, 4).reshape(128, BL * HALO * IMG)
        )
        in_maps.append(
            dict(xp=xpk, ws=ws, qs=qs, w1=w1, w2=w2, b1=b1, b2=b2)
        )
    return in_maps


def kernel(x, w, q, fc1_w, fc1_b, fc2_w, fc2_b):
    if "nc" not in _CACHE:
        _CACHE["nc"] = _build_nc()
    nc = _CACHE["nc"]
    in_maps = _prep_inputs(x, w, q, fc1_w, fc1_b, fc2_w, fc2_b)
    # The axon-tunneled devices occasionally throw a transient
    # NRT_EXEC_UNIT_UNRECOVERABLE on the first execution of a fresh NEFF;
    # a retry has always succeeded with identical results.
    last_err = None
    for attempt in range(3):
        try:
            res = run_bass_kernel_spmd(nc, in_maps, list(range(NCORES)))
            break
        except Exception as e:  # noqa: BLE001 - retry transient device faults
            last_err = e
            import time as _time
            _time.sleep(5 * (attempt + 1))
    else:
        raise last_err
    _CACHE["last_exec_time_ns"] = res.exec_time_ns
    _CACHE["last_results"] = res
    outp = np.empty((B, 10), np.float32)
    for k in range(NCORES):
        o = np.asarray(res.results[k]["out"], np.float32)   # [10, BL]
        outp[BL * k : BL * k + BL, :] = o.T
    return outp



# revision 3
# speedup vs baseline: 1.0402x; 1.0402x over previous
"""Trainium2 Bass kernel for the modified-MDPN dendrite model (v2).

Math per output element (b, i, j, m):
    acc = sum_r log(prod_c u)  with  u = atan(10*(x*w - q))/pi + 1.1
        = log(prod_{r,c} u)    (u > 0 always)
Then 4x4 spatial maxpool, flatten (io, jo, m), fc1(7744->128)+relu,
fc2(128->10).

Device strategy (8 NeuronCores, data parallel over batch, 2 images/core):
  - partitions p = m*8 + cp (m: 16 filters, cp: 8 chunks of 11 output
    rows; 8*11 = 88 = S exactly, so the atan stream carries ZERO padding:
    free size per tap is 2 images * 11 rows * 88 cols = 1936 elems (the
    v1 kernel used 12-row chunks = 2112 elems to keep the 4x4 maxpool
    in-partition; v2 moves the row pool across partitions instead).
  - per tap (r, c): one ACT Arctan over [128, 1936] with per-partition
    scale=10w, bias=-10q folded into the activation pre-affine; one DVE
    tensor_scalar (u = t/pi + 1.1, bf16, 4x mode); one DVE tensor_tensor
    multiply into a running product (bf16, 2x mode, ping-pong).  Taps run
    r-major so the input DMA (rows stream in order) stays ahead.
  - ln is monotonic, so the 4x4 maxpool runs on the bf16 *products*:
    col-pool (groups of 4 j) is a free-dim reduce; the row pool crosses
    the 11-row chunks, so the tiny j-pooled map ([128, 24*11*2] bf16) is
    repacked by 8 SBUF->SBUF DMAs onto partitions p' = m*8 + joc
    (joc: 8 chunks of 3 jo columns; 2 of the 24 jo slots are zero-pad,
    masked by zero fc1 weights), free dim (jos, i, b) -- then the row
    pool is 3 pairwise DVE max ops fully inside partitions.
  - Ln never runs on ACT (avoids Arctan<->Ln table reloads, 1283ns each):
    ln(v) ~= ln2*(bits(v)/128 - 127 + 0.043) via a bf16->int16 bitcast +
    one DVE tensor_scalar.  Max approx error 0.030 on pooled values,
    ~1e-3 relative on the logits (measured total stays ~4e-3).
  - fc1: the pooled map y2ln is [(m,joc) partitions, (jos,ig,b)], so the
    7744-long contraction runs directly as 66 accumulating K=128 matmuls
    (rhs [128, 2] contiguous in b) against host-permuted bf16 weights;
    relu+bias and the fc2 bias run on DVE (no ACT table traffic); fc2 is
    one matmul.
  - startup: ws/qs first, then x rows in 6 small slices (tap 0 is split
    into 6 matching pieces) so the first atan starts right after the
    Arctan table load; the big fc1 weights DMA last.  Tail: the last tap
    is split into 4 pool-aligned j-quarters so the product multiplies and
    j-pools pipeline against the final atans.

Engine balance per core (cost model): ACT 81 taps * ~1705ns = ~138us
stream (critical path), DVE ~130us, PE ~1us.  v1 measured 175.8us on the
same cost model; v2 removes the 8.3% row padding, 2 of 3 activation
table loads, and ~7us of startup/tail stalls.
"""

import math
import sys

sys.path.insert(0, "/opt/trn_rl_repo")

import ml_dtypes
import numpy as np

import concourse.bacc as bacc
import concourse.mybir as mybir
from concourse import tile
from concourse.bass_utils import run_bass_kernel_spmd

AFT = mybir.ActivationFunctionType
ALU = mybir.AluOpType
F32 = mybir.dt.float32
BF16 = mybir.dt.bfloat16
I16 = mybir.dt.int16

M = 16          # filters
N = 9           # window side
IMG = 96
S = 88          # sliding-window output side
SP = 22         # pooled side
B = 16          # global batch
NCORES = 8
BL = B // NCORES          # images per core (2)
CP = 8                    # row chunks per image (11 rows each)
RP = 11                   # output rows per chunk (8*11 = 88, no pad)
HALO = RP + N - 1         # input rows per chunk (19)
FD = BL * RP * S          # free elems per tap instruction (1936)
JOP = 24                  # jo slots incl 2 zero-pads (8 joc * 3 jos)
PI = float(np.pi)
# ln(v) ~= LN_S0 * int16_bits(bf16 v) + LN_S1  (positive normal v)
LN_S0 = math.log(2.0) / 128.0
LN_S1 = math.log(2.0) * (0.043 - 127.0)

_CACHE = {}


def _build_nc():
    nc = bacc.Bacc("TRN2", target_bir_lowering=False, debug=False)

    xp = nc.declare_dram_parameter("xp", [128, BL * HALO * IMG], F32, isOutput=False)
    ws = nc.declare_dram_parameter("ws", [128, 81], F32, isOutput=False)
    qs = nc.declare_dram_parameter("qs", [128, 81], F32, isOutput=False)
    w1 = nc.declare_dram_parameter("w1", [128, 66 * 128], BF16, isOutput=False)
    w2 = nc.declare_dram_parameter("w2", [128, 10], F32, isOutput=False)
    b1 = nc.declare_dram_parameter("b1", [128, 1], F32, isOutput=False)
    b2 = nc.declare_dram_parameter("b2", [10, 1], F32, isOutput=False)
    out = nc.declare_dram_parameter("out", [10, BL], F32, isOutput=True)

    with tile.TileContext(nc) as tc:
        with (
            tc.tile_pool(name="consts", bufs=1) as cpool,
            tc.tile_pool(name="work", bufs=3) as wpool,
            tc.tile_pool(name="state", bufs=1) as spool,
            tc.tile_pool(name="psum", bufs=1, space="PSUM") as ppool,
        ):
            xs = cpool.tile([128, BL * HALO * IMG], F32, tag="xs")
            wst = cpool.tile([128, 81], F32, tag="wst")
            qst = cpool.tile([128, 81], F32, tag="qst")
            w1t = cpool.tile([128, 66 * 128], BF16, tag="w1t")
            w2t = cpool.tile([128, 10], F32, tag="w2t")
            b1t = cpool.tile([128, 1], F32, tag="b1t")
            b2t = cpool.tile([10, 1], F32, tag="b2t")

            xsr = xs[:].rearrange("p (b il j) -> p b il j", b=BL, il=HALO, j=IMG)
            xpr = xp.rearrange("p (b il j) -> p b il j", b=BL, il=HALO, j=IMG)

            # DMA order matters: the first atans wait on ws/qs and the
            # first row slices; the big fc1 weights go last.  Tap 0 is
            # split into 6 pieces matching the first 6 slices.
            nc.sync.dma_start(wst[:], ws[:])
            nc.sync.dma_start(qst[:], qs[:])
            head = [(0, 0, 4), (0, 4, 8), (0, 8, RP),
                    (1, 0, 4), (1, 4, 8), (1, 8, RP)]
            for bb, il0, il1 in head:
                nc.sync.dma_start(
                    xsr[:, bb : bb + 1, il0:il1], xpr[:, bb : bb + 1, il0:il1]
                )
            for bb in range(BL):
                nc.sync.dma_start(
                    xsr[:, bb : bb + 1, RP:HALO], xpr[:, bb : bb + 1, RP:HALO]
                )
            nc.sync.dma_start(b1t[:], b1[:])
            nc.sync.dma_start(b2t[:], b2[:])
            nc.sync.dma_start(w2t[:], w2[:])
            nc.sync.dma_start(w1t[:], w1[:])

            # j-pooled map, layout (jo, il, b); jo slots 22..23 stay zero.
            p1 = spool.tile([128, JOP * RP * BL], BF16, tag="p1")
            nc.vector.memset(p1[:], 0.0)
            p1v = p1[:].rearrange(
                "p (jo il b) -> p b il jo", jo=JOP, il=RP, b=BL
            )

            rp_tiles = [
                spool.tile([128, FD], BF16, tag="rp0", name="rp0"),
                spool.tile([128, FD], BF16, tag="rp1", name="rp1"),
            ]
            cur = 0

            def affine(dst, src):
                nc.vector.tensor_scalar(
                    dst, src, 1.0 / PI, 1.1, ALU.mult, ALU.add
                )

            # tap 0 in 6 pieces (affine writes the product tile directly)
            p0v = rp_tiles[0][:].rearrange(
                "p (b il j) -> p b il j", b=BL, il=RP, j=S
            )
            for bb, il0, il1 in head:
                utp = wpool.tile([128, 1, il1 - il0, S], BF16, tag="atan")
                nc.scalar.activation(
                    utp[:], xsr[:, bb : bb + 1, il0:il1, 0:S], AFT.Arctan,
                    bias=qst[:, 0:1], scale=wst[:, 0:1],
                )
                affine(p0v[:, bb : bb + 1, il0:il1], utp[:])

            # taps 1..79: full-size stream
            for t in range(1, 80):
                r, c = divmod(t, N)
                xv = xsr[:, :, r : r + RP, c : c + S]
                ut = wpool.tile([128, BL, RP, S], BF16, tag="atan")
                nc.scalar.activation(
                    ut[:], xv, AFT.Arctan,
                    bias=qst[:, t : t + 1], scale=wst[:, t : t + 1],
                )
                un = wpool.tile([128, FD], BF16, tag="un")
                affine(un[:], ut[:].rearrange("p b il j -> p (b il j)"))
                nxt = 1 - cur
                nc.vector.tensor_tensor(
                    rp_tiles[nxt][:], rp_tiles[cur][:], un[:], ALU.mult
                )
                cur = nxt

            # tap 80 in 4 pool-aligned j-quarters so mult + j-pool pipeline
            # against the last atans; j-pool for earlier quarters runs while
            # ACT finishes the later ones.
            r, c = divmod(80, N)
            jq = [(0, 5), (5, 11), (11, 16), (16, 22)]   # jo ranges
            fin = rp_tiles[1 - cur]
            finv = fin[:].rearrange("p (b il j) -> p b il j", b=BL, il=RP, j=S)
            curv = rp_tiles[cur][:].rearrange(
                "p (b il j) -> p b il j", b=BL, il=RP, j=S
            )
            finq = fin[:].rearrange(
                "p (b il jo jj) -> p b il jo jj", b=BL, il=RP, jo=SP, jj=4
            )
            for q0, q1 in jq:
                j0, j1 = 4 * q0, 4 * q1
                utq = wpool.tile([128, BL, RP, j1 - j0], BF16, tag="atan")
                nc.scalar.activation(
                    utq[:], xsr[:, :, r : r + RP, c + j0 : c + j1], AFT.Arctan,
                    bias=qst[:, 80:81], scale=wst[:, 80:81],
                )
                unq = wpool.tile([128, BL * RP * (j1 - j0)], BF16, tag="un")
                affine(unq[:], utq[:].rearrange("p b il j -> p (b il j)"))
                nc.vector.tensor_tensor(
                    finv[:, :, :, j0:j1],
                    curv[:, :, :, j0:j1],
                    unq[:].rearrange(
                        "p (b il j) -> p b il j", b=BL, il=RP, j=j1 - j0
                    ),
                    ALU.mult,
                )
                # col pool for this quarter (free-dim reduce over jj)
                nc.vector.tensor_reduce(
                    p1v[:, :, :, q0:q1],
                    finq[:, :, :, q0:q1, :],
                    mybir.AxisListType.X,
                    ALU.max,
                )

            # repack (transpose-DMA): partitions (m, cp) -> (m, joc),
            # free (jos, i, b); 8 small SBUF->SBUF DMAs, one per cp.
            p1T = spool.tile([128, 3 * S * BL], BF16, tag="p1T")
            p1s = p1[:].rearrange(
                "(m cp) (jo il b) -> m cp jo il b",
                m=M, cp=CP, jo=JOP, il=RP, b=BL,
            )
            p1Tv = p1T[:].rearrange(
                "(m joc) (jos i b) -> m joc jos i b",
                m=M, joc=CP, jos=3, i=S, b=BL,
            )
            for cp in range(CP):
                srcv = p1s[:, cp].rearrange(
                    "m (joc jos) il b -> m joc jos il b", joc=CP, jos=3
                )
                dstv = p1Tv[:, :, :, RP * cp : RP * cp + RP, :].rearrange(
                    "m joc jos il b -> (m joc) jos il b"
                )
                nc.sync.dma_start(dstv, srcv)

            # row pool: 3 pairwise max ops, all inside partitions now
            p1Ti = p1T[:].rearrange(
                "p (jos ig ii b) -> p jos ig ii b", jos=3, ig=SP, ii=4, b=BL
            )
            t0 = spool.tile([128, 3 * SP * BL], BF16, tag="t0")
            t1 = spool.tile([128, 3 * SP * BL], BF16, tag="t1")
            y2 = spool.tile([128, 3 * SP * BL], BF16, tag="y2")
            t0v = t0[:].rearrange("p (jos ig b) -> p jos ig b", jos=3, ig=SP, b=BL)
            t1v = t1[:].rearrange("p (jos ig b) -> p jos ig b", jos=3, ig=SP, b=BL)
            nc.vector.tensor_tensor(
                t0v, p1Ti[:, :, :, 0], p1Ti[:, :, :, 1], ALU.max
            )
            nc.vector.tensor_tensor(
                t1v, p1Ti[:, :, :, 2], p1Ti[:, :, :, 3], ALU.max
            )
            nc.vector.tensor_tensor(y2[:], t0[:], t1[:], ALU.max)

            # dendrite output: ln of the pooled product via bf16-bits trick
            y2ln = spool.tile([128, 3 * SP * BL], BF16, tag="y2ln")
            nc.vector.tensor_scalar(
                y2ln[:], y2[:].bitcast(I16), LN_S0, LN_S1, ALU.mult, ALU.add
            )

            # fc1: contraction over all 7744 = 128 partitions x 66 groups
            ph = ppool.tile([128, BL], F32, tag="ph")
            y2f = y2ln[:].rearrange("p (g b) -> p g b", g=66, b=BL)
            for g in range(66):
                nc.tensor.matmul(
                    ph[:],
                    w1t[:, g * 128 : (g + 1) * 128],
                    y2f[:, g],
                    start=(g == 0),
                    stop=(g == 65),
                )
            # relu(ph + b1) on DVE (keeps ACT free of table traffic)
            h = spool.tile([128, BL], F32, tag="h")
            nc.vector.tensor_scalar(
                h[:], ph[:], b1t[:], 0.0, ALU.add, ALU.max
            )

            # fc2 + bias
            po = ppool.tile([10, BL], F32, tag="po")
            nc.tensor.matmul(po[:], w2t[:, 0:10], h[:], start=True, stop=True)
            osb = spool.tile([10, BL], F32, tag="osb")
            nc.vector.tensor_scalar(
                osb[:], po[:], b2t[:], 0.0, ALU.add, ALU.add
            )
            nc.sync.dma_start(out[:], osb[:])

    nc.compile()
    return nc


def _prep_inputs(x, w, q, fc1_w, fc1_b, fc2_w, fc2_b):
    x = np.asarray(x, np.float32)
    w = np.asarray(w, np.float32)
    q = np.asarray(q, np.float32)
    fc1_w = np.asarray(fc1_w, np.float32)
    fc1_b = np.asarray(fc1_b, np.float32)
    fc2_w = np.asarray(fc2_w, np.float32)
    fc2_b = np.asarray(fc2_b, np.float32)

    # halo chunks: [B, CP, HALO, IMG]; 11*7 + 19 = 96 exactly, no padding
    xh = np.stack(
        [x[:, RP * cp : RP * cp + HALO, :] for cp in range(CP)], axis=1
    )

    ws = np.repeat(10.0 * w.reshape(M, 81), CP, axis=0)          # [128, 81]
    qs = np.repeat(-10.0 * q.reshape(M, 81), CP, axis=0)

    # fc1 weights: w1[(m*8+joc) partition, (jos*22+ig) group, n]
    #   = fc1_w[n, ig*352 + jo*16 + m] with jo = 3*joc + jos (0 if jo >= 22)
    A = fc1_w.reshape(128, SP, SP, M)             # [n, io(=ig), jo, m]
    A2 = A.transpose(3, 2, 1, 0)                  # [m, jo, ig, n]
    A3 = np.zeros((M, JOP, SP, 128), np.float32)
    A3[:, :SP] = A2
    w1 = np.ascontiguousarray(
        A3.reshape(M, CP, 3, SP, 128).reshape(128, 66 * 128)
    ).astype(ml_dtypes.bfloat16)

    w2 = np.ascontiguousarray(fc2_w.T)            # [128, 10]
    b1 = fc1_b.reshape(128, 1).astype(np.float32)
    b2 = fc2_b.reshape(10, 1).astype(np.float32)

    in_maps = []
    for k in range(NCORES):
        arr = xh[BL * k : BL * k + BL]            # [BL, CP, HALO, IMG]
        xpk = np.broadcast_to(arr[None], (M, BL, CP, HALO, IMG))
        xpk = np.ascontiguousarray(
            xpk.transpose(0, 2, 1, 3, 4).reshape(128, BL * HALO * IMG)
        )
        in_maps.append(
            dict(xp=xpk, ws=ws, qs=qs, w1=w1, w2=w2, b1=b1, b2=b2)
        )
    return in_maps


def kernel(x, w, q, fc1_w, fc1_b, fc2_w, fc2_b):
    if "nc" not in _CACHE:
        _CACHE["nc"] = _build_nc()
    nc = _CACHE["nc"]
    in_maps = _prep_inputs(x, w, q, fc1_w, fc1_b, fc2_w, fc2_b)
    # The axon-tunneled devices occasionally throw a transient
    # NRT_EXEC_UNIT_UNRECOVERABLE on the first execution of a fresh NEFF;
    # a retry has always succeeded with identical results.
    last_err = None
    for attempt in range(3):
        try:
            res = run_bass_kernel_spmd(nc, in_maps, list(range(NCORES)))
            break
        except Exception as e:  # noqa: BLE001 - retry transient device faults
            last_err = e
            import time as _time
            _time.sleep(5 * (attempt + 1))
    else:
        raise last_err
    _CACHE["last_exec_time_ns"] = res.exec_time_ns
    _CACHE["last_results"] = res
    outp = np.empty((B, 10), np.float32)
    for k in range(NCORES):
        o = np.asarray(res.results[k]["out"], np.float32)   # [10, BL]
        outp[BL * k : BL * k + BL, :] = o.T
    return outp


# revision 4
# speedup vs baseline: 1.0575x; 1.0166x over previous
"""Trainium2 Bass kernel for the modified-MDPN dendrite model (v3).

Math per output element (b, i, j, m):
    acc = sum_r log(prod_c u)  with  u = atan(10*(x*w - q))/pi + 1.1
        = log(prod_{r,c} u)    (u > 0 always)
Then 4x4 spatial maxpool, flatten (io, jo, m), fc1(7744->128)+relu,
fc2(128->10).

Device strategy (8 NeuronCores, data parallel over batch, 2 images/core):
  - partitions p = 4*(b*16 + m) + cp (b: 2 images, m: 16 filters, cp: 4
    chunks of 22 output rows; 4*22 = 88 = S exactly, so the atan stream
    carries ZERO padding: free size per tap is 22 rows * 88 cols = 1936).
  - per tap (r, c): one ACT Arctan over [128, 1936] with per-partition
    scale=10w, bias=-10q folded into the activation pre-affine; one DVE
    tensor_scalar (u = t/pi + 1.1, bf16, 4x mode); one DVE tensor_tensor
    multiply into a running product (bf16, 2x mode, ping-pong).  Taps run
    r-major so the input DMA (rows stream in order) stays ahead; tap 0 is
    split into 4 row-pieces matched to the first DMA slices.
  - ln is monotonic, so the 4x4 maxpool runs on the bf16 *products*:
    col-pool (groups of 4 j) is a free-dim tensor_reduce; the row pool
    crosses the 22-row chunks, so the tiny j-pooled map ([128, 24*22]
    bf16, 2 of 24 jo slots zero-pad) is repacked by FOUR SBUF->SBUF DMAs
    (one per cp; each DMA costs ~650ns fixed HWDGE time, which is why cp=4
    beats cp=8) onto partitions p' = 4*(b*16+m) + joc, free (jos, i) --
    then the row pool is one in-partition tensor_reduce.
  - Ln never runs on ACT (avoids Arctan<->Ln table reloads, 1283ns each):
    ln(v) ~= ln2*(bits(v)/128 - 127 + 0.043) via a bf16->int16 bitcast +
    one DVE tensor_scalar.  ~1e-3 relative on the logits.
  - fc1: contraction over (m, jo, ig) runs as 132 accumulating K=128
    matmuls; batch lives on partitions, so the rhs is a host-zeroed
    two-column copy of y2ln (col b nonzero only on image-b partitions),
    giving both images' hidden vectors in one PSUM [128, 2].  relu+bias
    and the fc2 bias run on DVE; fc2 is one matmul.
  - tail: the last tap runs in 4 pool-aligned jo-quarters with all
    affine+mults issued before the j-pools so DVE drains fast; the four
    repack DMAs alternate between the SP and ACT HWDGE queues.

Engine balance per core (cost model): ACT 81 taps * ~1705ns = ~138us
stream (critical path), DVE ~131us, PE ~1us.  v1 measured 175.8us, v2
169.0us on the same cost model.
"""

import math
import sys

sys.path.insert(0, "/opt/trn_rl_repo")

import ml_dtypes
import numpy as np

import concourse.bacc as bacc
import concourse.mybir as mybir
from concourse import tile
from concourse.bass_utils import run_bass_kernel_spmd

AFT = mybir.ActivationFunctionType
ALU = mybir.AluOpType
F32 = mybir.dt.float32
BF16 = mybir.dt.bfloat16
I16 = mybir.dt.int16

M = 16          # filters
N = 9           # window side
IMG = 96
S = 88          # sliding-window output side
SP = 22         # pooled side
B = 16          # global batch
NCORES = 8
BL = B // NCORES          # images per core (2)
CP = 4                    # row chunks (22 rows each; 4*22 = 88, no pad)
RP = 22                   # output rows per chunk
HALO = RP + N - 1         # input rows per chunk (30)
FD = RP * S               # free elems per tap instruction (1936)
JOP = 24                  # jo slots incl 2 zero-pads (4 joc * 6 jos)
G1 = 6 * SP               # fc1 groups (132)
PI = float(np.pi)
# ln(v) ~= LN_S0 * int16_bits(bf16 v) + LN_S1  (positive normal v)
LN_S0 = math.log(2.0) / 128.0
LN_S1 = math.log(2.0) * (0.043 - 127.0)

_CACHE = {}


def _build_nc():
    nc = bacc.Bacc("TRN2", target_bir_lowering=False, debug=False)

    xp = nc.declare_dram_parameter("xp", [128, HALO * IMG], F32, isOutput=False)
    wq = nc.declare_dram_parameter("wq", [128, 162], F32, isOutput=False)
    w1 = nc.declare_dram_parameter("w1", [128, G1 * 128], BF16, isOutput=False)
    w2b = nc.declare_dram_parameter("w2b", [128, 11], F32, isOutput=False)
    b2 = nc.declare_dram_parameter("b2", [10, 1], F32, isOutput=False)
    out = nc.declare_dram_parameter("out", [10, BL], F32, isOutput=True)

    with tile.TileContext(nc) as tc:
        with (
            tc.tile_pool(name="consts", bufs=1) as cpool,
            tc.tile_pool(name="work", bufs=3) as wpool,
            tc.tile_pool(name="state", bufs=1) as spool,
            tc.tile_pool(name="psum", bufs=1, space="PSUM") as ppool,
        ):
            xs = cpool.tile([128, HALO * IMG], F32, tag="xs")
            wqt = cpool.tile([128, 162], F32, tag="wqt")
            w1t = cpool.tile([128, G1 * 128], BF16, tag="w1t")
            w2bt = cpool.tile([128, 11], F32, tag="w2bt")
            b2t = cpool.tile([10, 1], F32, tag="b2t")
            wst = wqt[:, 0:81]
            qst = wqt[:, 81:162]

            xsr = xs[:].rearrange("p (il j) -> p il j", il=HALO, j=IMG)
            xpr = xp.rearrange("p (il j) -> p il j", il=HALO, j=IMG)

            # ws/qs on the ACT HWDGE queue (runs in parallel with the SP
            # queue's first x slice); x row slices sized so tap 0's pieces
            # start as early as possible; big fc1 weights last.
            nc.scalar.dma_start(wqt[:], wq[:])
            head = [(0, 3), (3, 8), (8, 14), (14, RP)]
            for il0, il1 in head:
                nc.sync.dma_start(xsr[:, il0:il1], xpr[:, il0:il1])
            nc.sync.dma_start(xsr[:, RP:HALO], xpr[:, RP:HALO])
            nc.sync.dma_start(b2t[:], b2[:])
            nc.sync.dma_start(w2bt[:], w2b[:])
            nc.sync.dma_start(w1t[:], w1[:])

            # j-pooled map, layout (jo, il); jo slots 22..23 stay zero.
            p1 = spool.tile([128, JOP * RP], BF16, tag="p1")
            nc.vector.memset(p1[:], 0.0)
            p1v = p1[:].rearrange("p (jo il) -> p il jo", jo=JOP, il=RP)
            # image-masked two-column rhs for fc1 (memset covers the
            # opposite-image zeros once)
            y2m = spool.tile([128, G1 * BL], BF16, tag="y2m")
            nc.vector.memset(y2m[:], 0.0)

            rp_tiles = [
                spool.tile([128, FD], BF16, tag="rp0", name="rp0"),
                spool.tile([128, FD], BF16, tag="rp1", name="rp1"),
            ]
            cur = 0

            def affine(dst, src):
                nc.vector.tensor_scalar(
                    dst, src, 1.0 / PI, 1.1, ALU.mult, ALU.add
                )

            # tap 0 in 4 row-pieces (affine writes the product tile directly)
            p0v = rp_tiles[0][:].rearrange("p (il j) -> p il j", il=RP, j=S)
            for il0, il1 in head:
                utp = wpool.tile([128, il1 - il0, S], BF16, tag="atan")
                nc.scalar.activation(
                    utp[:], xsr[:, il0:il1, 0:S], AFT.Arctan,
                    bias=qst[:, 0:1], scale=wst[:, 0:1],
                )
                affine(p0v[:, il0:il1], utp[:])

            # taps 1..79: full-size stream
            for t in range(1, 80):
                r, c = divmod(t, N)
                xv = xsr[:, r : r + RP, c : c + S]
                ut = wpool.tile([128, RP, S], BF16, tag="atan")
                nc.scalar.activation(
                    ut[:], xv, AFT.Arctan,
                    bias=qst[:, t : t + 1], scale=wst[:, t : t + 1],
                )
                un = wpool.tile([128, FD], BF16, tag="un")
                affine(un[:], ut[:].rearrange("p il j -> p (il j)"))
                nxt = 1 - cur
                nc.vector.tensor_tensor(
                    rp_tiles[nxt][:], rp_tiles[cur][:], un[:], ALU.mult
                )
                cur = nxt

            # tap 80 in 4 pool-aligned jo-quarters; affine+mults lead, the
            # j-pools trail so DVE drains quickly after the last atan.
            r, c = divmod(80, N)
            jq = [(0, 5), (5, 11), (11, 16), (16, 22)]   # jo ranges
            fin = rp_tiles[1 - cur]
            finv = fin[:].rearrange("p (il j) -> p il j", il=RP, j=S)
            curv = rp_tiles[cur][:].rearrange("p (il j) -> p il j", il=RP, j=S)
            finq = fin[:].rearrange(
                "p (il jo jj) -> p il jo jj", il=RP, jo=SP, jj=4
            )
            jpools = []
            for q0, q1 in jq:
                j0, j1 = 4 * q0, 4 * q1
                utq = wpool.tile([128, RP, j1 - j0], BF16, tag="atan")
                nc.scalar.activation(
                    utq[:], xsr[:, r : r + RP, c + j0 : c + j1], AFT.Arctan,
                    bias=qst[:, 80:81], scale=wst[:, 80:81],
                )
                unq = wpool.tile([128, RP * (j1 - j0)], BF16, tag="un")
                affine(unq[:], utq[:].rearrange("p il j -> p (il j)"))
                nc.vector.tensor_tensor(
                    finv[:, :, j0:j1],
                    curv[:, :, j0:j1],
                    unq[:].rearrange("p (il j) -> p il j", il=RP, j=j1 - j0),
                    ALU.mult,
                )
                jpools.append((q0, q1))
                if len(jpools) >= 2:
                    qq0, qq1 = jpools.pop(0)
                    nc.vector.tensor_reduce(
                        p1v[:, :, qq0:qq1],
                        finq[:, :, qq0:qq1, :],
                        mybir.AxisListType.X,
                        ALU.max,
                    )
            for qq0, qq1 in jpools:
                nc.vector.tensor_reduce(
                    p1v[:, :, qq0:qq1],
                    finq[:, :, qq0:qq1, :],
                    mybir.AxisListType.X,
                    ALU.max,
                )

            # repack (transpose-DMA): partitions (k, cp) -> (k, joc),
            # free (jos, i); 4 small SBUF->SBUF DMAs alternating queues.
            p1T = spool.tile([128, 6 * S], BF16, tag="p1T")
            p1s = p1[:].rearrange(
                "(kk cp2) (joc jos il) -> kk cp2 joc jos il",
                kk=32, cp2=CP, joc=CP, jos=6, il=RP,
            )
            p1Tv = p1T[:].rearrange(
                "(kk joc) (jos i) -> kk joc jos i", kk=32, joc=CP, jos=6, i=S
            )
            for cp in range(CP):
                eng = nc.sync if cp % 2 == 0 else nc.scalar
                eng.dma_start(
                    p1Tv[:, :, :, RP * cp : RP * cp + RP].rearrange(
                        "kk joc jos il -> (kk joc) jos il"
                    ),
                    p1s[:, cp],
                )

            # row pool (in-partition now): one reduce over ii
            p1Ti = p1T[:].rearrange(
                "p (jos ig ii) -> p jos ig ii", jos=6, ig=SP, ii=4
            )
            y2 = spool.tile([128, G1], BF16, tag="y2")
            nc.vector.tensor_reduce(
                y2[:].rearrange("p (jos ig) -> p jos ig", jos=6, ig=SP),
                p1Ti, mybir.AxisListType.X, ALU.max,
            )

            # dendrite output: ln of the pooled product via bf16-bits trick
            y2l = spool.tile([128, G1], BF16, tag="y2l")
            nc.vector.tensor_scalar(
                y2l[:], y2[:].bitcast(I16), LN_S0, LN_S1, ALU.mult, ALU.add
            )
            # scatter into the image-masked two-column rhs
            y2mv = y2m[:].rearrange("p (g c) -> p g c", g=G1, c=BL)
            nc.vector.tensor_scalar(
                y2mv[0:64, :, 0], y2l[0:64, :], 1.0, 0.0, ALU.mult, ALU.add
            )
            nc.vector.tensor_scalar(
                y2mv[64:128, :, 1], y2l[64:128, :], 1.0, 0.0, ALU.mult, ALU.add
            )

            # fc1: 132 accumulating K=128 matmuls, both images per matmul
            ph = ppool.tile([128, BL], F32, tag="ph")
            for g in range(G1):
                nc.tensor.matmul(
                    ph[:],
                    w1t[:, g * 128 : (g + 1) * 128],
                    y2mv[:, g],
                    start=(g == 0),
                    stop=(g == G1 - 1),
                )
            # relu(ph + b1) on DVE (keeps ACT free of table traffic)
            h = spool.tile([128, BL], F32, tag="h")
            nc.vector.tensor_scalar(
                h[:], ph[:], w2bt[:, 10:11], 0.0, ALU.add, ALU.max
            )

            # fc2 + bias
            po = ppool.tile([10, BL], F32, tag="po")
            nc.tensor.matmul(
                po[:], w2bt[:, 0:10], h[:], start=True, stop=True
            )
            osb = spool.tile([10, BL], F32, tag="osb")
            nc.vector.tensor_scalar(
                osb[:], po[:], b2t[:], 0.0, ALU.add, ALU.add
            )
            nc.sync.dma_start(out[:], osb[:])

    nc.compile()
    return nc


def _prep_inputs(x, w, q, fc1_w, fc1_b, fc2_w, fc2_b):
    x = np.asarray(x, np.float32)
    w = np.asarray(w, np.float32)
    q = np.asarray(q, np.float32)
    fc1_w = np.asarray(fc1_w, np.float32)
    fc1_b = np.asarray(fc1_b, np.float32)
    fc2_w = np.asarray(fc2_w, np.float32)
    fc2_b = np.asarray(fc2_b, np.float32)

    # halo chunks: [B, CP, HALO, IMG]; 22*3 + 30 = 96 exactly, no padding
    xh = np.stack(
        [x[:, RP * cp : RP * cp + HALO, :] for cp in range(CP)], axis=1
    )

    # ws/qs rows follow p = 4*(b*16+m) + cp
    wm = np.repeat(10.0 * w.reshape(M, 81), CP, axis=0)      # [64, 81]
    qm = np.repeat(-10.0 * q.reshape(M, 81), CP, axis=0)
    wq = np.hstack([np.tile(wm, (BL, 1)), np.tile(qm, (BL, 1))])  # [128,162]
    wq = np.ascontiguousarray(wq)

    # fc1 weights: w1[(k*4+joc) partition, (jos*22+ig) group, n]
    #   = fc1_w[n, ig*352 + jo*16 + m], jo = 6*joc + jos (0 if jo >= 22);
    # independent of the image half (k = b*16 + m).
    A = fc1_w.reshape(128, SP, SP, M)             # [n, ig, jo, m]
    A2 = A.transpose(3, 2, 1, 0)                  # [m, jo, ig, n]
    A3 = np.zeros((M, JOP, SP, 128), np.float32)
    A3[:, :SP] = A2
    half = A3.reshape(M, CP, 6, SP, 128).reshape(64, G1 * 128)
    w1 = np.ascontiguousarray(np.tile(half, (BL, 1))).astype(ml_dtypes.bfloat16)

    w2b = np.hstack([fc2_w.T, fc1_b.reshape(128, 1)]).astype(np.float32)
    w2b = np.ascontiguousarray(w2b)
    b2 = fc2_b.reshape(10, 1).astype(np.float32)

    in_maps = []
    for k in range(NCORES):
        arr = xh[BL * k : BL * k + BL]            # [BL, CP, HALO, IMG]
        xpk = np.broadcast_to(arr[:, None], (BL, M, CP, HALO, IMG))
        xpk = np.ascontiguousarray(xpk.reshape(128, HALO * IMG))
        in_maps.append(dict(xp=xpk, wq=wq, w1=w1, w2b=w2b, b2=b2))
    return in_maps


def kernel(x, w, q, fc1_w, fc1_b, fc2_w, fc2_b):
    if "nc" not in _CACHE:
        _CACHE["nc"] = _build_nc()
    nc = _CACHE["nc"]
    in_maps = _prep_inputs(x, w, q, fc1_w, fc1_b, fc2_w, fc2_b)
    # The axon-tunneled devices occasionally throw a transient
    # NRT_EXEC_UNIT_UNRECOVERABLE on the first execution of a fresh NEFF;
    # a retry has always succeeded with identical results.
    last_err = None
    for attempt in range(3):
        try:
            res = run_bass_kernel_spmd(nc, in_maps, list(range(NCORES)))
            break
        except Exception as e:  # noqa: BLE001 - retry transient device faults
            last_err = e
            import time as _time
            _time.sleep(5 * (attempt + 1))
    else:
        raise last_err
    _CACHE["last_exec_time_ns"] = res.exec_time_ns
    _CACHE["last_results"] = res
    outp = np.empty((B, 10), np.float32)
    for k in range(NCORES):
        o = np.asarray(res.results[k]["out"], np.float32)   # [10, BL]
        outp[BL * k : BL * k + BL, :] = o.T
    return outp


# revision 16
# speedup vs baseline: 1.0732x; 1.0149x over previous
"""Trainium2 Bass kernel for the modified-MDPN dendrite model (v3).

Math per output element (b, i, j, m):
    acc = sum_r log(prod_c u)  with  u = atan(10*(x*w - q))/pi + 1.1
        = log(prod_{r,c} u)    (u > 0 always)
Then 4x4 spatial maxpool, flatten (io, jo, m), fc1(7744->128)+relu,
fc2(128->10).

Device strategy (8 NeuronCores, data parallel over batch, 2 images/core):
  - partitions p = 4*(b*16 + m) + cp (b: 2 images, m: 16 filters, cp: 4
    chunks of 22 output rows; 4*22 = 88 = S exactly, so the atan stream
    carries ZERO padding: free size per tap is 22 rows * 88 cols = 1936).
  - per tap (r, c): one ACT Arctan over [128, 1936] with per-partition
    scale=10w, bias=-10q folded into the activation pre-affine; one DVE
    tensor_scalar (u = t/pi + 1.1, bf16, 4x mode); one DVE tensor_tensor
    multiply into a running product (bf16, 2x mode, ping-pong).  Taps run
    r-major so the input DMA (rows stream in order) stays ahead; tap 0 is
    split into 4 row-pieces matched to the first DMA slices.
  - ln is monotonic, so the 4x4 maxpool runs on the bf16 *products*:
    col-pool (groups of 4 j) is a free-dim tensor_reduce; the row pool
    crosses the 22-row chunks, so the tiny j-pooled map ([128, 24*22]
    bf16, 2 of 24 jo slots zero-pad) is repacked by FOUR SBUF->SBUF DMAs
    (one per cp; each DMA costs ~650ns fixed HWDGE time, which is why cp=4
    beats cp=8) onto partitions p' = 4*(b*16+m) + joc, free (jos, i) --
    then the row pool is one in-partition tensor_reduce.
  - Ln never runs on ACT (avoids Arctan<->Ln table reloads, 1283ns each):
    ln(v) ~= ln2*(bits(v)/128 - 127 + 0.043) via a bf16->int16 bitcast +
    one DVE tensor_scalar.  ~1e-3 relative on the logits.
  - fc1: contraction over (m, jo, ig) runs as 132 accumulating K=128
    matmuls; batch lives on partitions, so the rhs is a host-zeroed
    two-column copy of y2ln (col b nonzero only on image-b partitions),
    giving both images' hidden vectors in one PSUM [128, 2].  relu+bias
    and the fc2 bias run on DVE; fc2 is one matmul.
  - tail: the last tap runs in 4 pool-aligned jo-quarters with all
    affine+mults issued before the j-pools so DVE drains fast; the four
    repack DMAs alternate between the SP and ACT HWDGE queues.

Engine balance per core (cost model): ACT 81 taps * ~1705ns = ~138us
stream (critical path), DVE ~131us, PE ~1us.  v1 measured 175.8us, v2
169.0us on the same cost model.
"""

import math
import sys

sys.path.insert(0, "/opt/trn_rl_repo")

import ml_dtypes
import numpy as np

import concourse.bacc as bacc
import concourse.mybir as mybir
from concourse import tile
from concourse.bass_utils import run_bass_kernel_spmd

AFT = mybir.ActivationFunctionType
ALU = mybir.AluOpType
F32 = mybir.dt.float32
BF16 = mybir.dt.bfloat16
I16 = mybir.dt.int16

M = 16          # filters
N = 9           # window side
IMG = 96
S = 88          # sliding-window output side
SP = 22         # pooled side
B = 16          # global batch
NCORES = 8
BL = B // NCORES          # images per core (2)
CP = 4                    # row chunks (22 rows each; 4*22 = 88, no pad)
RP = 22                   # output rows per chunk
HALO = RP + N - 1         # input rows per chunk (30)
FD = RP * S               # free elems per tap instruction (1936)
JOP = 24                  # jo slots incl 2 zero-pads (4 joc * 6 jos)
G1 = 4 * 6 * 6            # fc1 groups (cp, jos, slot) incl garbage slots
IO0 = (0, 6, 11, 17)      # first row-group owned by chunk cp
OFF = (0, 2, 0, 2)        # in-chunk il offset of the first owned group
PI = float(np.pi)
# ln(v) ~= LN_S0 * int16_bits(bf16 v) + LN_S1  (positive normal v)
LN_S0 = math.log(2.0) / 128.0
LN_S1 = math.log(2.0) * (0.043 - 127.0)

_CACHE = {}


def _build_nc():
    nc = bacc.Bacc("TRN2", target_bir_lowering=False, debug=False)

    xp = nc.declare_dram_parameter("xp", [128, HALO * IMG], BF16, isOutput=False)
    wq = nc.declare_dram_parameter("wq", [128, 162], F32, isOutput=False)
    w1 = nc.declare_dram_parameter("w1", [128, G1 * 128], BF16, isOutput=False)
    w2b = nc.declare_dram_parameter("w2b", [128, 11], F32, isOutput=False)
    b2 = nc.declare_dram_parameter("b2", [10, 1], F32, isOutput=False)
    out = nc.declare_dram_parameter("out", [10, BL], F32, isOutput=True)

    with tile.TileContext(nc) as tc:
        with (
            tc.tile_pool(name="consts", bufs=1) as cpool,
            tc.tile_pool(name="work", bufs=3) as wpool,
            tc.tile_pool(name="state", bufs=1) as spool,
            tc.tile_pool(name="psum", bufs=1, space="PSUM") as ppool,
        ):
            xs = cpool.tile([128, HALO * IMG], BF16, tag="xs")
            wqt = cpool.tile([128, 162], F32, tag="wqt")
            w1t = cpool.tile([128, G1 * 128], BF16, tag="w1t")
            w2bt = cpool.tile([128, 11], F32, tag="w2bt")
            b2t = cpool.tile([10, 1], F32, tag="b2t")
            wst = wqt[:, 0:81]
            qst = wqt[:, 81:162]

            xsr = xs[:].rearrange("p (il j) -> p il j", il=HALO, j=IMG)
            xpr = xp.rearrange("p (il j) -> p il j", il=HALO, j=IMG)

            # ws/qs on the ACT HWDGE queue (runs in parallel with the SP
            # queue's first x slice); x row slices sized so tap 0's pieces
            # start as early as possible; big fc1 weights last.
            nc.scalar.dma_start(wqt[:], wq[:])
            head = [(0, 6), (6, 14), (14, RP)]
            for il0, il1 in head:
                nc.sync.dma_start(xsr[:, il0:il1], xpr[:, il0:il1])
            nc.sync.dma_start(xsr[:, RP:HALO], xpr[:, RP:HALO])
            nc.sync.dma_start(b2t[:], b2[:])
            nc.sync.dma_start(w2bt[:], w2b[:])
            nc.sync.dma_start(w1t[:], w1[:])

            # j-pooled map, layout (jo, il); jo slots 22..23 stay zero.
            p1 = spool.tile([128, JOP * RP], BF16, tag="p1")
            nc.vector.memset(p1[:], 0.0)
            p1v = p1[:].rearrange("p (jo il) -> p il jo", jo=JOP, il=RP)
            # image-masked two-column rhs for fc1 (memset covers the
            # opposite-image zeros once)
            y2m = spool.tile([128, G1 * BL], BF16, tag="y2m")
            nc.vector.memset(y2m[:], 0.0)
            # pooled map incl garbage slots (odd-cp slot 5, zero-weighted)
            y2g = spool.tile([128, G1], BF16, tag="y2")
            nc.vector.memset(y2g[:], 1.0)

            rp_tiles = [
                spool.tile([128, FD], BF16, tag="rp0", name="rp0"),
                spool.tile([128, FD], BF16, tag="rp1", name="rp1"),
            ]
            cur = 0

            def affine(dst, src):
                nc.vector.tensor_scalar(
                    dst, src, 1.0 / PI, 1.1, ALU.mult, ALU.add
                )

            # tap 0 in 4 row-pieces (affine writes the product tile directly)
            p0v = rp_tiles[0][:].rearrange("p (il j) -> p il j", il=RP, j=S)
            for il0, il1 in head:
                utp = wpool.tile([128, il1 - il0, S], BF16, tag="atan")
                nc.scalar.activation(
                    utp[:], xsr[:, il0:il1, 0:S], AFT.Arctan,
                    bias=qst[:, 0:1], scale=wst[:, 0:1],
                )
                affine(p0v[:, il0:il1], utp[:])

            # taps 1..78: full-size stream
            for t in range(1, 79):
                r, c = divmod(t, N)
                xv = xsr[:, r : r + RP, c : c + S]
                ut = wpool.tile([128, RP, S], BF16, tag="atan")
                nc.scalar.activation(
                    ut[:], xv, AFT.Arctan,
                    bias=qst[:, t : t + 1], scale=wst[:, t : t + 1],
                )
                un = wpool.tile([128, FD], BF16, tag="un")
                affine(un[:], ut[:].rearrange("p il j -> p (il j)"))
                nxt = 1 - cur
                nc.vector.tensor_tensor(
                    rp_tiles[nxt][:], rp_tiles[cur][:], un[:], ALU.mult
                )
                cur = nxt

            # tap 79 in j-halves (halves the ACT->DVE pipeline skew going
            # into the tail)
            r, c = divmod(79, N)
            nxt = 1 - cur
            nv = rp_tiles[nxt][:].rearrange("p (il j) -> p il j", il=RP, j=S)
            cv = rp_tiles[cur][:].rearrange("p (il j) -> p il j", il=RP, j=S)
            for j0, j1 in [(0, 44), (44, S)]:
                uth = wpool.tile([128, RP, j1 - j0], BF16, tag="atan")
                nc.scalar.activation(
                    uth[:], xsr[:, r : r + RP, c + j0 : c + j1], AFT.Arctan,
                    bias=qst[:, 79:80], scale=wst[:, 79:80],
                )
                unh = wpool.tile([128, RP * (j1 - j0)], BF16, tag="un")
                affine(unh[:], uth[:].rearrange("p il j -> p (il j)"))
                nc.vector.tensor_tensor(
                    nv[:, :, j0:j1], cv[:, :, j0:j1],
                    unh[:].rearrange("p (il j) -> p il j", il=RP, j=j1 - j0),
                    ALU.mult,
                )
            cur = nxt

            # tap 80 in 4 joc-aligned jo-quarters; each quarter's j-pool
            # feeds its own repack DMA immediately (per-joc DMAs overlap
            # the remaining quarters' compute).  Repack: partitions
            # (k, cp) -> (k, joc), free (cp, jos, il) -- this free order
            # keeps both DMA access patterns mergeable to <= 3 dims.
            p1T = spool.tile([128, CP * 6 * RP], BF16, tag="p1T")
            p1j = p1[:].rearrange("p (jo il) -> p jo il", jo=JOP, il=RP)
            p1Td = p1T[:].rearrange(
                "(kk joc) (cp jos il) -> kk joc cp jos il",
                kk=32, joc=CP, jos=6, cp=CP, il=RP,
            )
            r, c = divmod(80, N)
            jq = [(0, 6), (6, 12), (12, 18), (18, 22)]   # jo ranges
            fin = rp_tiles[1 - cur]
            finv = fin[:].rearrange("p (il j) -> p il j", il=RP, j=S)
            curv = rp_tiles[cur][:].rearrange("p (il j) -> p il j", il=RP, j=S)
            finq = fin[:].rearrange(
                "p (il jo jj) -> p il jo jj", il=RP, jo=SP, jj=4
            )
            for qi, (q0, q1) in enumerate(jq):
                j0, j1 = 4 * q0, 4 * q1
                utq = wpool.tile([128, RP, j1 - j0], BF16, tag="atan")
                nc.scalar.activation(
                    utq[:], xsr[:, r : r + RP, c + j0 : c + j1], AFT.Arctan,
                    bias=qst[:, 80:81], scale=wst[:, 80:81],
                )
                unq = wpool.tile([128, RP * (j1 - j0)], BF16, tag="un")
                affine(unq[:], utq[:].rearrange("p il j -> p (il j)"))
                nc.vector.tensor_tensor(
                    finv[:, :, j0:j1],
                    curv[:, :, j0:j1],
                    unq[:].rearrange("p (il j) -> p il j", il=RP, j=j1 - j0),
                    ALU.mult,
                )
                nc.vector.tensor_reduce(
                    p1v[:, :, q0:q1],
                    finq[:, :, q0:q1, :],
                    mybir.AxisListType.X,
                    ALU.max,
                )
                # repack DMA for this joc block (jo 6*qi..6*qi+6; block 3
                # includes the memset zero-pad columns 22..23).  Quarters
                # 0-2 issue from the idle SP queue; the last from ACT,
                # which is free once its atan above has issued.
                eng = nc.sync if qi < 3 else nc.scalar
                eng.dma_start(
                    p1Td[:, qi],
                    p1j[:, 6 * qi : 6 * qi + 6, :],
                )

            # row pool over global row-groups, slot-based: chunk cp owns
            # row-groups io = IO0[cp] + s (6 slots for even cp, 5 + garbage
            # for odd cp); the two groups that cross a chunk boundary
            # (io 5 and 16) are finished with a tiny max against the next
            # chunk's first two rows -- all in free dims, no extra DMA.
            p1Tc = p1T[:].rearrange(
                "p (ce par jos il) -> p ce par jos il",
                ce=2, par=2, jos=6, il=RP,
            )
            y2 = y2g
            y2v = y2[:].rearrange(
                "p (ce par jos s) -> p ce par jos s", ce=2, par=2, jos=6, s=6
            )
            for par in (0, 1):   # chunk parity: il offset 0 (even) / 2 (odd)
                off = OFF[par]
                ein = p1Tc[:, :, par, :, off : off + 4 * 5].rearrange(
                    "p ce jos (s ii) -> p ce jos s ii", s=5, ii=4
                )
                nc.vector.tensor_reduce(
                    y2v[:, :, par, :, 0:5], ein, mybir.AxisListType.X, ALU.max
                )
            # even-cp slot 5: rows il 20:22 ...
            nc.vector.tensor_reduce(
                y2v[:, :, 0, :, 5],
                p1Tc[:, :, 0, :, 20:22],
                mybir.AxisListType.X,
                ALU.max,
            )
            # ... maxed with rows il 0:2 of the following (odd) chunk
            t5 = spool.tile([128, 2 * 6], BF16, tag="t5")
            t5v = t5[:].rearrange("p (ce jos) -> p ce jos", ce=2, jos=6)
            nc.vector.tensor_reduce(
                t5v, p1Tc[:, :, 1, :, 0:2], mybir.AxisListType.X, ALU.max
            )
            nc.vector.tensor_tensor(
                y2v[:, :, 0, :, 5], y2v[:, :, 0, :, 5], t5v, ALU.max
            )

            # ln of the pooled product via the bf16-bits trick, folded into
            # the scatter to the image-masked two-column fc1 rhs
            y2b = y2[:].bitcast(I16)
            y2mv = y2m[:].rearrange("p (g c) -> p g c", g=G1, c=BL)
            nc.vector.tensor_scalar(
                y2mv[0:64, :, 0], y2b[0:64, :], LN_S0, LN_S1, ALU.mult, ALU.add
            )
            nc.vector.tensor_scalar(
                y2mv[64:128, :, 1], y2b[64:128, :], LN_S0, LN_S1,
                ALU.mult, ALU.add
            )

            # fc1: 132 accumulating K=128 matmuls, both images per matmul
            ph = ppool.tile([128, BL], F32, tag="ph")
            for g in range(G1):
                nc.tensor.matmul(
                    ph[:],
                    w1t[:, g * 128 : (g + 1) * 128],
                    y2mv[:, g],
                    start=(g == 0),
                    stop=(g == G1 - 1),
                )
            # relu(ph + b1) on DVE (keeps ACT free of table traffic)
            h = spool.tile([128, BL], F32, tag="h")
            nc.vector.tensor_scalar(
                h[:], ph[:], w2bt[:, 10:11], 0.0, ALU.add, ALU.max
            )

            # fc2 + bias
            po = ppool.tile([10, BL], F32, tag="po")
            nc.tensor.matmul(
                po[:], w2bt[:, 0:10], h[:], start=True, stop=True
            )
            osb = spool.tile([10, BL], F32, tag="osb")
            nc.vector.tensor_scalar(
                osb[:], po[:], b2t[:], 0.0, ALU.add, ALU.add
            )
            nc.sync.dma_start(out[:], osb[:])

    nc.compile()
    return nc


def _prep_inputs(x, w, q, fc1_w, fc1_b, fc2_w, fc2_b):
    x = np.asarray(x, np.float32)
    w = np.asarray(w, np.float32)
    q = np.asarray(q, np.float32)
    fc1_w = np.asarray(fc1_w, np.float32)
    fc1_b = np.asarray(fc1_b, np.float32)
    fc2_w = np.asarray(fc2_w, np.float32)
    fc2_b = np.asarray(fc2_b, np.float32)

    # halo chunks: [B, CP, HALO, IMG]; 22*3 + 30 = 96 exactly, no padding
    xh = np.stack(
        [x[:, RP * cp : RP * cp + HALO, :] for cp in range(CP)], axis=1
    )

    # ws/qs rows follow p = 4*(b*16+m) + cp
    wm = np.repeat(10.0 * w.reshape(M, 81), CP, axis=0)      # [64, 81]
    qm = np.repeat(-10.0 * q.reshape(M, 81), CP, axis=0)
    wq = np.hstack([np.tile(wm, (BL, 1)), np.tile(qm, (BL, 1))])  # [128,162]
    wq = np.ascontiguousarray(wq)

    # fc1 weights: w1[(k*4+joc) partition, (cp, jos, s) group, n]
    #   = fc1_w[n, io*352 + jo*16 + m], jo = 6*joc + jos, io = IO0[cp] + s;
    # zero for jo >= 22 and for the garbage slots (odd cp, s == 5);
    # independent of the image half (k = b*16 + m).
    A = fc1_w.reshape(128, SP, SP, M)             # [n, io, jo, m]
    W = np.zeros((M, CP, CP, 6, 6, 128), np.float32)  # [m,joc,cp,jos,s,n]
    for joc in range(CP):
        for jos in range(6):
            jo = 6 * joc + jos
            if jo >= SP:
                continue
            for cp in range(CP):
                for s in range(6):
                    if s == 5 and cp % 2 == 1:
                        continue
                    io = IO0[cp] + s
                    W[:, joc, cp, jos, s, :] = A[:, io, jo, :].T
    half = W.reshape(64, G1 * 128)
    w1 = np.ascontiguousarray(np.tile(half, (BL, 1))).astype(ml_dtypes.bfloat16)

    w2b = np.hstack([fc2_w.T, fc1_b.reshape(128, 1)]).astype(np.float32)
    w2b = np.ascontiguousarray(w2b)
    b2 = fc2_b.reshape(10, 1).astype(np.float32)

    in_maps = []
    for k in range(NCORES):
        arr = xh[BL * k : BL * k + BL]            # [BL, CP, HALO, IMG]
        xpk = np.broadcast_to(arr[:, None], (BL, M, CP, HALO, IMG))
        xpk = np.ascontiguousarray(xpk.reshape(128, HALO * IMG)).astype(
            ml_dtypes.bfloat16
        )
        in_maps.append(dict(xp=xpk, wq=wq, w1=w1, w2b=w2b, b2=b2))
    return in_maps


def kernel(x, w, q, fc1_w, fc1_b, fc2_w, fc2_b):
    if "nc" not in _CACHE:
        _CACHE["nc"] = _build_nc()
    nc = _CACHE["nc"]
    in_maps = _prep_inputs(x, w, q, fc1_w, fc1_b, fc2_w, fc2_b)
    # The axon-tunneled devices occasionally throw a transient
    # NRT_EXEC_UNIT_UNRECOVERABLE on the first execution of a fresh NEFF;
    # a retry has always succeeded with identical results.
    last_err = None
    for attempt in range(3):
        try:
            res = run_bass_kernel_spmd(nc, in_maps, list(range(NCORES)))
            break
        except Exception as e:  # noqa: BLE001 - retry transient device faults
            last_err = e
            import time as _time
            _time.sleep(5 * (attempt + 1))
    else:
        raise last_err
    _CACHE["last_exec_time_ns"] = res.exec_time_ns
    _CACHE["last_results"] = res
    outp = np.empty((B, 10), np.float32)
    for k in range(NCORES):
        o = np.asarray(res.results[k]["out"], np.float32)   # [10, BL]
        outp[BL * k : BL * k + BL, :] = o.T
    return outp
